# revision 2
# baseline (speedup 1.0000x reference)
"""Trainium2 Bass kernel for AGRNN edge-MLP message passing.

Math (per edge e):
    feat = [node_feat[dst], node_feat_lang[dst], edge_spatial[e],
            node_feat_lang[src], node_feat[src]]            # [1640]
    pred[e] = relu(feat @ W1 + b1) @ W2 + b2                # [13]

Strategy (8 NeuronCores, pure edge parallelism, no collectives):
  - Host sorts each core's edges by dst (ascending gather addresses;
    output un-permuted host-side) and compacts the packed node table
    per (core, phase): phase A = groups 0-24, phase B = 25-48; each
    phase's dst+src refs are deduplicated (<= 25600 uniques < 2^15,
    since the ucode dma_gather treats int16 indices as SIGNED) and
    remapped.
  - Node row layout [896 bf16]: [vis 512 | lang 300 | 1.0 | zeros] —
    the 1.0 column folds b1 into W1 (dst side); 1792 B rows satisfy
    dma_gather's 256 B elem-size granularity.
  - The `mlp` GPSIMD ucode dma_gather(transpose=True) fetches 512 rows
    per instruction straight into feature-major layout [128, 7, 512] —
    no PE transposes at all (the v1 kernel burned ~180us of PE on
    them). num_idxs > 512 crashes the runtime; 512 is the sweet spot.
  - Gathers round-robin over 4 SWDGE queues (num_swdge_queues=4, ISA
    queue_num), which lifts random-row SDMA drain from ~160 GB/s
    (single queue, the v1 INDIRECT1D wall) to ~300 GB/s.
  - edge_spatial is host-transposed and DMA'd (ACT HWDGE queue) over
    the dst-side pad partitions (chunk 6, partitions 48:64); W1 rows
    816:832 carry the s_f weights.
  - W1 runs in h^T form: W1 chunk [128f,128h] stationary, 512 edges
    stream per matmul (14 accumulating MMs x 2 h-chunks per group);
    ACT applies relu off the h^T PSUM; W2 consumes h^T directly; b2 is
    folded into the DVE PSUM->SBUF copy as a per-partition scalar add.
    Output f32 [13, 25088] per core; host transposes, trims, unsorts.

Measured (8-core SPMD, all cores identical; worst-core time):
  ~405us HW exec, rel err 3.3e-3 (v1 baseline: 587us).  Busy:
  TensorMatrix ~340us (W1 stream 309 + W2 31), GpSimd ~312us
  (98 dma_gathers, incl. ring backpressure), PE idle ~70us
  (startup/pipeline fill; warmup tricks moved the stall, net 0).
  Remaining walls are balanced: PE W1 (bf16 floor ~292us), SDMA
  drain ~300us, Q7 desc-gen ~290us.
"""

import os

import numpy as np
import ml_dtypes

import concourse.bass as bass
import concourse.mybir as mybir
from concourse import bacc
from concourse.tile import TileContext

BF16 = mybir.dt.bfloat16
F32 = mybir.dt.float32
I16 = mybir.dt.int16

N_NODES = 100000
F_VIS = 512
F_LANG = 300
F_SPAT = 16
HID = 256
NCLS = 13
N_CORES = 8

ROW = 896                 # node row (bf16): 512 vis + 300 lang + 1.0 + 83 pad
ONE_COL = F_VIS + F_LANG  # 812: constant-1.0 column (bias trick)
ES_PART = 48              # es rows at chunk 6, partitions 48:64
NCH = ROW // 128          # 7 chunks per side
NMM = 2 * NCH             # 14 chunk-matmuls per group half
FEAT = NMM * 128          # 1792 packed W1 rows
TILE_E = 128
GE = 512                  # edges per group
E_PER = 25088             # 49 groups of 512 (25000 real + 88 pad)
N_GROUPS = E_PER // GE    # 49
PH_GROUPS = (25, 24)      # phase split: uniques <= 2*512*25 = 25600 < 2^15
PH_ROWS = 25600           # table rows per phase (padded)

bf = ml_dtypes.bfloat16


def build_nc(nq=2, gath_bufs=4, ni=GE, es_on_act=True):
    nc = bacc.Bacc(
        None, target_bir_lowering=False, debug=False, num_swdge_queues=nq
    )
    table_d = nc.declare_dram_parameter(
        "table", [2, PH_ROWS, ROW], BF16, isOutput=False
    )
    w1_d = nc.declare_dram_parameter("w1", [FEAT, HID], BF16, isOutput=False)
    w2_d = nc.declare_dram_parameter("w2", [HID, NCLS], BF16, isOutput=False)
    b2_d = nc.declare_dram_parameter("b2", [NCLS, 1], F32, isOutput=False)
    # indices wrapped for dma_gather: idx j of group g at [j%16 (replicated
    # across the 8 16-partition blocks), g, j//16]
    didx_d = nc.declare_dram_parameter(
        "didx", [128, N_GROUPS, GE // 16], I16, isOutput=False
    )
    sidx_d = nc.declare_dram_parameter(
        "sidx", [128, N_GROUPS, GE // 16], I16, isOutput=False
    )
    es_d = nc.declare_dram_parameter(
        "es", [16, N_GROUPS, GE], BF16, isOutput=False
    )
    out_d = nc.declare_dram_parameter("out", [NCLS, E_PER], F32, isOutput=True)

    with TileContext(nc) as tc:
        with (
            tc.tile_pool(name="const", bufs=1) as constp,
            tc.tile_pool(name="gd", bufs=gath_bufs) as gdp,
            tc.tile_pool(name="gs", bufs=gath_bufs) as gsp,
            tc.tile_pool(name="hh", bufs=4) as hp,
            tc.tile_pool(name="oo", bufs=2) as outp,
            tc.tile_pool(name="ps_h", bufs=4, space="PSUM") as psh,
            tc.tile_pool(name="ps_o", bufs=2, space="PSUM") as pso,
        ):
            w1_sb = constp.tile([128, NMM, HID], BF16)
            nc.sync.dma_start(
                out=w1_sb[:], in_=w1_d[:].rearrange("(c k) h -> k c h", k=128)
            )
            w2_sb = constp.tile([128, 2, NCLS], BF16)
            nc.sync.dma_start(
                out=w2_sb[:], in_=w2_d[:].rearrange("(c k) n -> k c n", k=128)
            )
            b2_sb = constp.tile([NCLS, 1], F32)
            nc.sync.dma_start(out=b2_sb[:], in_=b2_d[:])
            didx_sb = constp.tile([128, N_GROUPS, GE // 16], I16)
            sidx_sb = constp.tile([128, N_GROUPS, GE // 16], I16)
            nc.sync.dma_start(out=didx_sb[:], in_=didx_d[:])
            nc.sync.dma_start(out=sidx_sb[:], in_=sidx_d[:])

            for g in range(N_GROUPS):
                ph = 0 if g < PH_GROUPS[0] else 1
                dst = gdp.tile([128, NCH, GE], BF16, tag="gd")
                nc.gpsimd.dma_gather(
                    out_ap=dst[:, :, :],
                    in_ap=table_d[ph, :, :],
                    idxs_ap=didx_sb[:, g, :],
                    num_idxs=GE,
                    num_idxs_reg=GE,
                    elem_size=ROW,
                    transpose=True,
                    queue_num=(2 * g) % nq,
                )
                # edge-spatial lands in the dst pad partitions (W1 rows 816:832)
                es_eng = nc.scalar if es_on_act else nc.sync
                es_eng.dma_start(
                    out=dst[ES_PART : ES_PART + 16, 6, :], in_=es_d[:, g, :]
                )
                src = gsp.tile([128, NCH, GE], BF16, tag="gs")
                nc.gpsimd.dma_gather(
                    out_ap=src[:, :, :],
                    in_ap=table_d[ph, :, :],
                    idxs_ap=sidx_sb[:, g, :],
                    num_idxs=GE,
                    num_idxs_reg=GE,
                    elem_size=ROW,
                    transpose=True,
                    queue_num=(2 * g + 1) % nq,
                )
                h_sbs = []
                for hc in range(2):
                    hT_ps = psh.tile([128, GE], F32, tag="hT")
                    for c in range(NCH):
                        nc.tensor.matmul(
                            out=hT_ps[:],
                            lhsT=w1_sb[:, c, hc * 128 : (hc + 1) * 128],
                            rhs=dst[:, c, :],
                            start=(c == 0),
                            stop=False,
                        )
                    for c in range(NCH):
                        nc.tensor.matmul(
                            out=hT_ps[:],
                            lhsT=w1_sb[:, NCH + c, hc * 128 : (hc + 1) * 128],
                            rhs=src[:, c, :],
                            start=False,
                            stop=(c == NCH - 1),
                        )
                    h_sb = hp.tile([128, GE], BF16)
                    nc.scalar.activation(
                        out=h_sb[:], in_=hT_ps[:],
                        func=mybir.ActivationFunctionType.Relu,
                    )
                    h_sbs.append(h_sb)
                o_ps = pso.tile([NCLS, GE], F32)
                nc.tensor.matmul(
                    out=o_ps[:], lhsT=w2_sb[:, 0, :], rhs=h_sbs[0][:],
                    start=True, stop=False,
                )
                nc.tensor.matmul(
                    out=o_ps[:], lhsT=w2_sb[:, 1, :], rhs=h_sbs[1][:],
                    start=False, stop=True,
                )
                o_sb = outp.tile([NCLS, GE], F32)
                # b2 folded into the PSUM->SBUF copy (per-partition scalar add)
                nc.vector.tensor_scalar_add(
                    out=o_sb[:], in0=o_ps[:], scalar1=b2_sb[:, 0:1]
                )
                nc.sync.dma_start(
                    out=out_d[:, g * GE : (g + 1) * GE], in_=o_sb[:]
                )
    nc.finalize()
    return nc


def prep_shared(node_feat, node_feat_lang, W1, b1, W2, b2):
    table = np.zeros((N_NODES, ROW), dtype=bf)
    table[:, :F_VIS] = node_feat.astype(bf)
    table[:, F_VIS:ONE_COL] = node_feat_lang.astype(bf)
    table[:, ONE_COL] = bf(1.0)

    # Reference concat order: [dst_vis, dst_lang, s_f, src_lang, src_vis]
    # Orig W1 rows: dst_vis 0:512, dst_lang 512:812, s_f 812:828,
    #               src_lang 828:1128, src_vis 1128:1640
    # Packed rows (14 chunks of 128): dst chunks 0-6 = rows 0:896 with the
    # chunk-6 pad carrying b1 (812) and es (816:832); src chunks 7-13 =
    # rows 896:1792 = src cols 0:896 (src-side 1.0 column keeps weight 0).
    w1p = np.zeros((FEAT, HID), np.float32)
    w1p[0:F_VIS] = W1[0:F_VIS]                                # dst vis
    w1p[F_VIS:ONE_COL] = W1[F_VIS:ONE_COL]                    # dst lang
    w1p[ONE_COL] = b1                                         # bias row
    w1p[768 + ES_PART : 768 + ES_PART + F_SPAT] = W1[812:828]   # s_f
    w1p[896:1408] = W1[1128:1640]                             # src vis (cols 0:512)
    w1p[1408:1708] = W1[828:1128]                             # src lang

    return {
        "table_full": table,
        "w1": w1p.astype(bf),
        "w2": W2.astype(bf),
        "b2": b2.reshape(NCLS, 1).astype(np.float32),
    }


def _wrap16(idx):
    """dma_gather index layout: idx j -> [j % 16, j // 16]."""
    return np.ascontiguousarray(idx.reshape(-1, 16).T).astype(np.int16)


def prep_core(table_full, didx, sidx, es):
    n = didx.shape[0]
    dpad = np.zeros(E_PER, np.int64)
    spad = np.zeros(E_PER, np.int64)
    dpad[:n] = didx
    spad[:n] = sidx

    table = np.zeros((2, PH_ROWS, ROW), dtype=bf)
    didx16 = np.zeros((16, N_GROUPS, GE // 16), np.int16)
    sidx16 = np.zeros((16, N_GROUPS, GE // 16), np.int16)
    # (replicated to 128 partitions below)
    g0 = 0
    for ph, ngr in enumerate(PH_GROUPS):
        lo, hi = g0 * GE, (g0 + ngr) * GE
        refs = np.concatenate([dpad[lo:hi], spad[lo:hi]])
        uniq, inv = np.unique(refs, return_inverse=True)
        assert uniq.size <= PH_ROWS
        table[ph, : uniq.size] = table_full[uniq]
        ne = hi - lo
        dl = inv[:ne].astype(np.int16).reshape(ngr, GE)
        sl = inv[ne:].astype(np.int16).reshape(ngr, GE)
        for j, g in enumerate(range(g0, g0 + ngr)):
            didx16[:, g, :] = _wrap16(dl[j])
            sidx16[:, g, :] = _wrap16(sl[j])
        g0 += ngr

    espad = np.zeros((E_PER, F_SPAT), np.float32)
    espad[:n] = es
    esT = np.ascontiguousarray(espad.T.reshape(F_SPAT, N_GROUPS, GE)).astype(bf)

    return {
        "table": table,
        "didx": np.ascontiguousarray(np.tile(didx16, (8, 1, 1))),
        "sidx": np.ascontiguousarray(np.tile(sidx16, (8, 1, 1))),
        "es": esT,
    }


_NC_CACHE = {}


def _get_nc():
    key = (
        int(os.environ.get("K_NQ", "4")),
        int(os.environ.get("K_GATH_BUFS", "6")),
        os.environ.get("K_ES_ACT", "1") == "1",
    )
    if key not in _NC_CACHE:
        _NC_CACHE[key] = build_nc(nq=key[0], gath_bufs=key[1], es_on_act=key[2])
    return _NC_CACHE[key]


def _install_trace_shim():
    import sys
    import types

    try:
        import antenv
        from trn_agent_boot.trn_boot import _ntff_profile_via_ctypes

        if "antenv.axon_hooks" not in sys.modules:
            mod = types.ModuleType("antenv.axon_hooks")
            mod._hook = None

            def set_axon_ntff_profile_hook(h):
                mod._hook = h

            def get_axon_ntff_profile_hook():
                return mod._hook

            mod.set_axon_ntff_profile_hook = set_axon_ntff_profile_hook
            mod.get_axon_ntff_profile_hook = get_axon_ntff_profile_hook
            sys.modules["antenv.axon_hooks"] = mod
            antenv.axon_hooks = mod
        hooks = sys.modules["antenv.axon_hooks"]
        if hooks.get_axon_ntff_profile_hook() is None:
            hooks.set_axon_ntff_profile_hook(
                _ntff_profile_via_ctypes("/opt/axon/libaxon_pjrt.so")
            )

        import concourse.bass_utils as bu

        bu.upload_artifacts = lambda tmpdir: f"local:{tmpdir}"
        return True
    except Exception as e:
        print(f"trace shim unavailable: {type(e).__name__}: {e}")
        return False


last_exec_time_ns = None
last_results = None


def kernel(**inputs):
    global last_exec_time_ns, last_results
    from concourse.bass_utils import run_bass_kernel_spmd

    node_feat = np.asarray(inputs["node_feat"], np.float32)
    node_feat_lang = np.asarray(inputs["node_feat_lang"], np.float32)
    edge_spatial = np.asarray(inputs["edge_spatial"], np.float32)
    W1 = np.asarray(inputs["W1"], np.float32)
    b1 = np.asarray(inputs["b1"], np.float32)
    W2 = np.asarray(inputs["W2"], np.float32)
    b2 = np.asarray(inputs["b2"], np.float32)
    src_idx = np.asarray(inputs["src_idx"]).astype(np.int64)
    dst_idx = np.asarray(inputs["dst_idx"]).astype(np.int64)

    E = dst_idx.shape[0]
    e_core = (E + N_CORES - 1) // N_CORES  # 25000

    shared = prep_shared(node_feat, node_feat_lang, W1, b1, W2, b2)
    table_full = shared.pop("table_full")

    sort = os.environ.get("K_SORT", "1") == "1"
    in_maps = []
    orders = []
    for c in range(N_CORES):
        lo, hi = c * e_core, min((c + 1) * e_core, E)
        d, s, e = dst_idx[lo:hi], src_idx[lo:hi], edge_spatial[lo:hi]
        if sort:
            # dst-sorted edge order: ascending gather addresses (HBM
            # locality); output un-permuted host-side below
            order = np.argsort(d, kind="stable")
            d, s, e = d[order], s[order], e[order]
        else:
            order = None
        orders.append(order)
        m = dict(shared)
        m.update(prep_core(table_full, d, s, e))
        in_maps.append(m)

    nc = _get_nc()
    trace = os.environ.get("KERNEL_TRACE", "0") == "1"
    if trace:
        _install_trace_shim()
    res = run_bass_kernel_spmd(
        nc, in_maps, core_ids=list(range(N_CORES)), trace=trace
    )
    last_exec_time_ns = res.exec_time_ns
    last_results = res

    out = np.empty((E, NCLS), np.float32)
    for c in range(N_CORES):
        lo, hi = c * e_core, min((c + 1) * e_core, E)
        o = res.results[c]["out"].T[: hi - lo]
        if orders[c] is not None:
            out[lo:hi][orders[c]] = o
        else:
            out[lo:hi] = o
    return out


# revision 5
# speedup vs baseline: 1.1405x; 1.1405x over previous
"""Trainium2 Bass kernel for AGRNN edge-MLP message passing.

Math (per edge e):
    feat = [node_feat[dst], node_feat_lang[dst], edge_spatial[e],
            node_feat_lang[src], node_feat[src]]            # [1640]
    pred[e] = relu(feat @ W1 + b1) @ W2 + b2                # [13]

Strategy (8 NeuronCores, pure edge parallelism, no collectives):
  - Host sorts each core's edges by dst (ascending gather addresses;
    output un-permuted host-side) and compacts the packed node table
    per (core, phase): phase A = groups 0-24, phase B = 25-48; each
    phase's dst+src refs are deduplicated (<= 25600 uniques < 2^15,
    since the ucode dma_gather treats int16 indices as SIGNED) and
    remapped.
  - Node row layout [896 bf16]: [vis 512 | lang 300 | 1.0 | zeros] —
    the 1.0 column folds b1 into W1 (dst side); 1792 B rows satisfy
    dma_gather's 256 B elem-size granularity.
  - The `mlp` GPSIMD ucode dma_gather(transpose=True) fetches 512 rows
    per instruction straight into feature-major layout [128, 7, 512] —
    no PE transposes at all (the v1 kernel burned ~180us of PE on
    them). num_idxs > 512 crashes the runtime; 512 is the sweet spot.
  - Gathers round-robin over 4 SWDGE queues (num_swdge_queues=4, ISA
    queue_num), which lifts random-row SDMA drain from ~160 GB/s
    (single queue, the v1 INDIRECT1D wall) to ~300 GB/s.
  - edge_spatial is host-transposed and DMA'd (ACT HWDGE queue) over
    the dst-side pad partitions (chunk 6, partitions 48:64); W1 rows
    816:832 carry the s_f weights.
  - W1 runs in h^T form: W1 chunk [128f,128h] stationary, 512 edges
    stream per matmul (14 accumulating MMs x 2 h-chunks per group);
    ACT applies relu off the h^T PSUM; W2 consumes h^T directly; b2 is
    folded into the DVE PSUM->SBUF copy as a per-partition scalar add.
    Output f32 [13, 25088] per core; host transposes, trims, unsorts.

Measured (8-core SPMD, all cores identical; worst-core time):
  405-470us HW exec across runs (device-state noise; best 404.7us),
  rel err 3.3e-3 (v1 baseline: 587us).  Busy at 405us:
  TensorMatrix ~340us (W1 stream 309 + W2 31), GpSimd ~312us
  (98 dma_gathers, incl. ring backpressure), PE idle ~70us
  (startup/pipeline fill; warmup tricks moved the stall, net 0).
  Remaining walls are balanced: PE W1 (bf16 floor ~292us), SDMA
  drain ~300us, Q7 desc-gen ~290us.
"""

import os

import numpy as np
import ml_dtypes

import concourse.bass as bass
import concourse.mybir as mybir
from concourse import bacc
from concourse.tile import TileContext

BF16 = mybir.dt.bfloat16
F32 = mybir.dt.float32
I16 = mybir.dt.int16

N_NODES = 100000
F_VIS = 512
F_LANG = 300
F_SPAT = 16
HID = 256
NCLS = 13
N_CORES = 8

ROW = 896                 # node row (bf16): 512 vis + 300 lang + 1.0 + 83 pad
ONE_COL = F_VIS + F_LANG  # 812: constant-1.0 column (bias trick)
ES_PART = 48              # es rows at chunk 6, partitions 48:64
NCH = ROW // 128          # 7 chunks per side
NMM = 2 * NCH             # 14 chunk-matmuls per group half
FEAT = NMM * 128          # 1792 packed W1 rows
TILE_E = 128
GE = 512                  # edges per group
E_PER = 25088             # 49 groups of 512 (25000 real + 88 pad)
N_GROUPS = E_PER // GE    # 49
PH_GROUPS = (25, 24)      # phase split: uniques <= 2*512*25 = 25600 < 2^15
PH_ROWS = 25600           # table rows per phase (padded)

bf = ml_dtypes.bfloat16


def build_nc(nq=2, gath_bufs=4, ni=GE, es_on_act=True):
    nc = bacc.Bacc(
        None, target_bir_lowering=False, debug=False, num_swdge_queues=nq
    )
    table_d = nc.declare_dram_parameter(
        "table", [2, PH_ROWS, ROW], BF16, isOutput=False
    )
    w1_d = nc.declare_dram_parameter("w1", [FEAT, HID], BF16, isOutput=False)
    w2_d = nc.declare_dram_parameter("w2", [HID, NCLS], BF16, isOutput=False)
    b2_d = nc.declare_dram_parameter("b2", [NCLS, 1], F32, isOutput=False)
    # indices wrapped for dma_gather: idx j of group g at [j%16 (replicated
    # across the 8 16-partition blocks), g, j//16]
    didx_d = nc.declare_dram_parameter(
        "didx", [128, N_GROUPS, GE // 16], I16, isOutput=False
    )
    sidx_d = nc.declare_dram_parameter(
        "sidx", [128, N_GROUPS, GE // 16], I16, isOutput=False
    )
    es_d = nc.declare_dram_parameter(
        "es", [16, N_GROUPS, GE], BF16, isOutput=False
    )
    out_d = nc.declare_dram_parameter("out", [NCLS, E_PER], F32, isOutput=True)

    with TileContext(nc) as tc:
        with (
            tc.tile_pool(name="const", bufs=1) as constp,
            tc.tile_pool(name="gd", bufs=gath_bufs) as gdp,
            tc.tile_pool(name="gs", bufs=gath_bufs) as gsp,
            tc.tile_pool(name="hh", bufs=4) as hp,
            tc.tile_pool(name="oo", bufs=2) as outp,
            tc.tile_pool(name="ps_h", bufs=6, space="PSUM") as psh,
            tc.tile_pool(name="ps_o", bufs=2, space="PSUM") as pso,
        ):
            w1_sb = constp.tile([128, NMM, HID], BF16)
            nc.sync.dma_start(
                out=w1_sb[:], in_=w1_d[:].rearrange("(c k) h -> k c h", k=128)
            )
            w2_sb = constp.tile([128, 2, NCLS], BF16)
            nc.sync.dma_start(
                out=w2_sb[:], in_=w2_d[:].rearrange("(c k) n -> k c n", k=128)
            )
            b2_sb = constp.tile([NCLS, 1], F32)
            nc.sync.dma_start(out=b2_sb[:], in_=b2_d[:])
            didx_sb = constp.tile([128, N_GROUPS, GE // 16], I16)
            sidx_sb = constp.tile([128, N_GROUPS, GE // 16], I16)
            nc.sync.dma_start(out=didx_sb[:], in_=didx_d[:])
            nc.sync.dma_start(out=sidx_sb[:], in_=sidx_d[:])

            for g in range(N_GROUPS):
                ph = 0 if g < PH_GROUPS[0] else 1
                dst = gdp.tile([128, NCH, GE], BF16, tag="gd")
                nc.gpsimd.dma_gather(
                    out_ap=dst[:, :, :],
                    in_ap=table_d[ph, :, :],
                    idxs_ap=didx_sb[:, g, :],
                    num_idxs=GE,
                    num_idxs_reg=GE,
                    elem_size=ROW,
                    transpose=True,
                    queue_num=(2 * g) % nq,
                )
                # edge-spatial lands in the dst pad partitions (W1 rows 816:832)
                es_eng = nc.scalar if es_on_act else nc.sync
                es_eng.dma_start(
                    out=dst[ES_PART : ES_PART + 16, 6, :], in_=es_d[:, g, :]
                )
                src = gsp.tile([128, NCH, GE], BF16, tag="gs")
                nc.gpsimd.dma_gather(
                    out_ap=src[:, :, :],
                    in_ap=table_d[ph, :, :],
                    idxs_ap=sidx_sb[:, g, :],
                    num_idxs=GE,
                    num_idxs_reg=GE,
                    elem_size=ROW,
                    transpose=True,
                    queue_num=(2 * g + 1) % nq,
                )
                h_sbs = []
                for hc in range(2):
                    hT_ps = psh.tile([128, GE], F32, tag="hT")
                    for c in range(NCH):
                        nc.tensor.matmul(
                            out=hT_ps[:],
                            lhsT=w1_sb[:, c, hc * 128 : (hc + 1) * 128],
                            rhs=dst[:, c, :],
                            start=(c == 0),
                            stop=False,
                        )
                    for c in range(NCH):
                        nc.tensor.matmul(
                            out=hT_ps[:],
                            lhsT=w1_sb[:, NCH + c, hc * 128 : (hc + 1) * 128],
                            rhs=src[:, c, :],
                            start=False,
                            stop=(c == NCH - 1),
                        )
                    h_sb = hp.tile([128, GE], BF16)
                    nc.scalar.activation(
                        out=h_sb[:], in_=hT_ps[:],
                        func=mybir.ActivationFunctionType.Relu,
                    )
                    h_sbs.append(h_sb)
                o_ps = pso.tile([NCLS, GE], F32)
                nc.tensor.matmul(
                    out=o_ps[:], lhsT=w2_sb[:, 0, :], rhs=h_sbs[0][:],
                    start=True, stop=False,
                )
                nc.tensor.matmul(
                    out=o_ps[:], lhsT=w2_sb[:, 1, :], rhs=h_sbs[1][:],
                    start=False, stop=True,
                )
                o_sb = outp.tile([NCLS, GE], F32)
                # b2 folded into the PSUM->SBUF copy (per-partition scalar add)
                nc.vector.tensor_scalar_add(
                    out=o_sb[:], in0=o_ps[:], scalar1=b2_sb[:, 0:1]
                )
                nc.sync.dma_start(
                    out=out_d[:, g * GE : (g + 1) * GE], in_=o_sb[:]
                )
    nc.finalize()
    return nc


def prep_shared(node_feat, node_feat_lang, W1, b1, W2, b2):
    table = np.zeros((N_NODES, ROW), dtype=bf)
    table[:, :F_VIS] = node_feat.astype(bf)
    table[:, F_VIS:ONE_COL] = node_feat_lang.astype(bf)
    table[:, ONE_COL] = bf(1.0)

    # Reference concat order: [dst_vis, dst_lang, s_f, src_lang, src_vis]
    # Orig W1 rows: dst_vis 0:512, dst_lang 512:812, s_f 812:828,
    #               src_lang 828:1128, src_vis 1128:1640
    # Packed rows (14 chunks of 128): dst chunks 0-6 = rows 0:896 with the
    # chunk-6 pad carrying b1 (812) and es (816:832); src chunks 7-13 =
    # rows 896:1792 = src cols 0:896 (src-side 1.0 column keeps weight 0).
    w1p = np.zeros((FEAT, HID), np.float32)
    w1p[0:F_VIS] = W1[0:F_VIS]                                # dst vis
    w1p[F_VIS:ONE_COL] = W1[F_VIS:ONE_COL]                    # dst lang
    w1p[ONE_COL] = b1                                         # bias row
    w1p[768 + ES_PART : 768 + ES_PART + F_SPAT] = W1[812:828]   # s_f
    w1p[896:1408] = W1[1128:1640]                             # src vis (cols 0:512)
    w1p[1408:1708] = W1[828:1128]                             # src lang

    return {
        "table_full": table,
        "w1": w1p.astype(bf),
        "w2": W2.astype(bf),
        "b2": b2.reshape(NCLS, 1).astype(np.float32),
    }


def _wrap16(idx):
    """dma_gather index layout: idx j -> [j % 16, j // 16]."""
    return np.ascontiguousarray(idx.reshape(-1, 16).T).astype(np.int16)


def prep_core(table_full, didx, sidx, es):
    n = didx.shape[0]
    dpad = np.zeros(E_PER, np.int64)
    spad = np.zeros(E_PER, np.int64)
    dpad[:n] = didx
    spad[:n] = sidx

    table = np.zeros((2, PH_ROWS, ROW), dtype=bf)
    didx16 = np.zeros((16, N_GROUPS, GE // 16), np.int16)
    sidx16 = np.zeros((16, N_GROUPS, GE // 16), np.int16)
    # (replicated to 128 partitions below)
    g0 = 0
    for ph, ngr in enumerate(PH_GROUPS):
        lo, hi = g0 * GE, (g0 + ngr) * GE
        refs = np.concatenate([dpad[lo:hi], spad[lo:hi]])
        uniq, inv = np.unique(refs, return_inverse=True)
        assert uniq.size <= PH_ROWS
        table[ph, : uniq.size] = table_full[uniq]
        ne = hi - lo
        dl = inv[:ne].astype(np.int16).reshape(ngr, GE)
        sl = inv[ne:].astype(np.int16).reshape(ngr, GE)
        for j, g in enumerate(range(g0, g0 + ngr)):
            didx16[:, g, :] = _wrap16(dl[j])
            sidx16[:, g, :] = _wrap16(sl[j])
        g0 += ngr

    espad = np.zeros((E_PER, F_SPAT), np.float32)
    espad[:n] = es
    esT = np.ascontiguousarray(espad.T.reshape(F_SPAT, N_GROUPS, GE)).astype(bf)

    return {
        "table": table,
        "didx": np.ascontiguousarray(np.tile(didx16, (8, 1, 1))),
        "sidx": np.ascontiguousarray(np.tile(sidx16, (8, 1, 1))),
        "es": esT,
    }


_NC_CACHE = {}


def _get_nc():
    key = (
        int(os.environ.get("K_NQ", "4")),
        int(os.environ.get("K_GATH_BUFS", "8")),
        os.environ.get("K_ES_ACT", "1") == "1",
    )
    if key not in _NC_CACHE:
        _NC_CACHE[key] = build_nc(nq=key[0], gath_bufs=key[1], es_on_act=key[2])
    return _NC_CACHE[key]


def _install_trace_shim():
    import sys
    import types

    try:
        import antenv
        from trn_agent_boot.trn_boot import _ntff_profile_via_ctypes

        if "antenv.axon_hooks" not in sys.modules:
            mod = types.ModuleType("antenv.axon_hooks")
            mod._hook = None

            def set_axon_ntff_profile_hook(h):
                mod._hook = h

            def get_axon_ntff_profile_hook():
                return mod._hook

            mod.set_axon_ntff_profile_hook = set_axon_ntff_profile_hook
            mod.get_axon_ntff_profile_hook = get_axon_ntff_profile_hook
            sys.modules["antenv.axon_hooks"] = mod
            antenv.axon_hooks = mod
        hooks = sys.modules["antenv.axon_hooks"]
        if hooks.get_axon_ntff_profile_hook() is None:
            hooks.set_axon_ntff_profile_hook(
                _ntff_profile_via_ctypes("/opt/axon/libaxon_pjrt.so")
            )

        import concourse.bass_utils as bu

        bu.upload_artifacts = lambda tmpdir: f"local:{tmpdir}"
        return True
    except Exception as e:
        print(f"trace shim unavailable: {type(e).__name__}: {e}")
        return False


last_exec_time_ns = None
last_results = None


def kernel(**inputs):
    global last_exec_time_ns, last_results
    from concourse.bass_utils import run_bass_kernel_spmd

    node_feat = np.asarray(inputs["node_feat"], np.float32)
    node_feat_lang = np.asarray(inputs["node_feat_lang"], np.float32)
    edge_spatial = np.asarray(inputs["edge_spatial"], np.float32)
    W1 = np.asarray(inputs["W1"], np.float32)
    b1 = np.asarray(inputs["b1"], np.float32)
    W2 = np.asarray(inputs["W2"], np.float32)
    b2 = np.asarray(inputs["b2"], np.float32)
    src_idx = np.asarray(inputs["src_idx"]).astype(np.int64)
    dst_idx = np.asarray(inputs["dst_idx"]).astype(np.int64)

    E = dst_idx.shape[0]
    e_core = (E + N_CORES - 1) // N_CORES  # 25000

    shared = prep_shared(node_feat, node_feat_lang, W1, b1, W2, b2)
    table_full = shared.pop("table_full")

    sort = os.environ.get("K_SORT", "1") == "1"
    in_maps = []
    orders = []
    for c in range(N_CORES):
        lo, hi = c * e_core, min((c + 1) * e_core, E)
        d, s, e = dst_idx[lo:hi], src_idx[lo:hi], edge_spatial[lo:hi]
        if sort:
            # dst-sorted edge order: ascending gather addresses (HBM
            # locality); output un-permuted host-side below
            order = np.argsort(d, kind="stable")
            d, s, e = d[order], s[order], e[order]
        else:
            order = None
        orders.append(order)
        m = dict(shared)
        m.update(prep_core(table_full, d, s, e))
        in_maps.append(m)

    nc = _get_nc()
    trace = os.environ.get("KERNEL_TRACE", "0") == "1"
    if trace:
        _install_trace_shim()
    res = run_bass_kernel_spmd(
        nc, in_maps, core_ids=list(range(N_CORES)), trace=trace
    )
    last_exec_time_ns = res.exec_time_ns
    last_results = res

    out = np.empty((E, NCLS), np.float32)
    for c in range(N_CORES):
        lo, hi = c * e_core, min((c + 1) * e_core, E)
        o = res.results[c]["out"].T[: hi - lo]
        if orders[c] is not None:
            out[lo:hi][orders[c]] = o
        else:
            out[lo:hi] = o
    return out
